# revision 27
# baseline (speedup 1.0000x reference)
"""Multi-head attention (B=4, S=2048, E=1024, 16 heads x 64) on 8 Trainium2 cores.

Sharding: core c = 2*b + half handles batch b and heads [8*half, 8*half+8)
(embed slice [512*half, 512*half+512)).  Each core computes its Q/K/V
projections, 8 heads of attention, and a row-parallel out-projection partial
(2048, 1024).  Host unshard: out[b] = partial[2b] + partial[2b+1] + bo.

Per-core device kernel (bf16 matmuls, fp32 accumulation):
  - QT/KT in [d_local, seq] layout (d on partitions) so energy^T = K @ Q^T
    comes out as [k_seq, q_seq] with softmax reductions computable by matmul.
  - softmax without max subtraction (energies are ~N(0,1); exp never overflows)
    with 1/sqrt(64) folded into Wq on the host.
  - exp on the scalar engine straight out of PSUM, bf16 output.
  - V carries an appended ones column so the attn@V matmul (M=65) yields the
    softmax denominator for free in PSUM row 64.
  - normalization: reciprocal of the sums row, gpsimd partition_broadcast,
    multiply-on-evict; odd heads are repacked to partitions 64:128 via
    SBUF->SBUF DMA so the out-projection contracts K=128 per matmul.
"""

import numpy as np
import ml_dtypes

import concourse.bass as bass
import concourse.mybir as mybir
import concourse.tile as tile
import concourse.bacc as bacc
from concourse.bass_utils import run_bass_kernel_spmd

BF16 = mybir.dt.bfloat16
F32 = mybir.dt.float32
NPBF = ml_dtypes.bfloat16

S = 2048          # sequence length
E = 1024          # embed dim
DLOC = 512        # per-core embed slice (8 heads x 64)
HD = 64           # head dim
NHL = 8           # heads per core
KT = E // 128     # 8 contraction tiles for projections
MT = DLOC // 128  # 4 m-tiles of d_local
ST = S // 128     # 16 seq tiles
NCH = S // 512    # 4 seq chunks of 512
EXP = mybir.ActivationFunctionType.Exp


def _build_bass(dump=False):
    nc = bacc.Bacc("TRN2", target_bir_lowering=False, debug=False)

    xqT = nc.dram_tensor("xqT", [E, S], BF16, kind="ExternalInput").ap()
    xkT = nc.dram_tensor("xkT", [E, S], BF16, kind="ExternalInput").ap()
    xvT = nc.dram_tensor("xvT", [E, S], BF16, kind="ExternalInput").ap()
    wq_d = nc.dram_tensor("wq", [E, DLOC], BF16, kind="ExternalInput").ap()
    wk_d = nc.dram_tensor("wk", [E, DLOC], BF16, kind="ExternalInput").ap()
    wv_d = nc.dram_tensor("wv", [E, DLOC], BF16, kind="ExternalInput").ap()
    wo_d = nc.dram_tensor("wo", [DLOC, E], BF16, kind="ExternalInput").ap()
    bq_d = nc.dram_tensor("bq", [128, MT], F32, kind="ExternalInput").ap()
    bk_d = nc.dram_tensor("bk", [128, MT], F32, kind="ExternalInput").ap()
    bv_d = nc.dram_tensor("bv", [1, DLOC], F32, kind="ExternalInput").ap()
    out_d = nc.dram_tensor("out", [S, E], F32, kind="ExternalOutput").ap()

    xq_r = xqT.rearrange("(kt p) s -> p kt s", p=128)
    xk_r = xkT.rearrange("(kt p) s -> p kt s", p=128)
    xv_r = xvT.rearrange("(kt p) s -> p kt s", p=128)

    with tile.TileContext(nc) as tc:
        _kernel_body(tc, nc, xq_r, xk_r, xv_r, wq_d, wk_d, wv_d, wo_d,
                     bq_d, bk_d, bv_d, out_d, dump=dump)
    nc.compile()
    return nc


def _kernel_body(tc, nc, xq_r, xk_r, xv_r, wq_d, wk_d, wv_d, wo_d,
                 bq_d, bk_d, bv_d, out_d, dump=False):
    from contextlib import ExitStack

    with ExitStack() as ctx:
        wpool = ctx.enter_context(tc.tile_pool(name="weights", bufs=1))
        xpool = ctx.enter_context(tc.tile_pool(name="xstream", bufs=6))
        qkv = ctx.enter_context(tc.tile_pool(name="qkv", bufs=1))
        atp = ctx.enter_context(tc.tile_pool(name="attnt", bufs=4))
        smp = ctx.enter_context(tc.tile_pool(name="small", bufs=2))
        outp = ctx.enter_context(tc.tile_pool(name="outstage", bufs=3))

        # ---- weights / biases to SBUF ----
        wq_sb = wpool.tile([128, KT, DLOC], BF16)
        wk_sb = wpool.tile([128, KT, DLOC], BF16)
        wv_sb = wpool.tile([128, KT, DLOC], BF16)
        wo_sb = wpool.tile([128, MT, E], BF16)
        bq_sb = wpool.tile([128, MT], F32)
        bk_sb = wpool.tile([128, MT], F32)
        bv_row = wpool.tile([1, DLOC], F32)
        bv_bc = wpool.tile([128, DLOC], F32)
        # (weight DMAs are interleaved with the prologue's x-chunk DMAs below
        # so the first K-projection isn't stuck behind wo/wv in the queue)
        nc.sync.dma_start(wk_sb[:], wk_d.rearrange("(kt p) m -> p kt m", p=128))
        nc.sync.dma_start(bk_sb[:], bk_d)

        # ---- persistent per-core tensors ----
        QT_sb = qkv.tile([128, MT, S], BF16)        # [d_loc, seq]
        KT_sb = qkv.tile([128, MT, S], BF16)
        V_sb = qkv.tile([128, ST, NHL, HD + 1], BF16)  # ones col at 64
        oT_sb = qkv.tile([128, MT, S], BF16)        # attn out^T (lhsT of outproj)

        nc.vector.memset(V_sb[:, :, :, HD:HD + 1], 1.0)

        # One PSUM layout for the whole kernel: 2x [128,1024] energy slots
        # (also used by proj/outproj psums) + 2x [65,1024] attn-out slots.
        pe_pool = ctx.enter_context(tc.tile_pool(name="psum_e", bufs=2, space="PSUM"))
        po_pool = ctx.enter_context(tc.tile_pool(name="psum_o", bufs=4, space="PSUM"))

        def x_dma(src_i, nch):
            x_t = xpool.tile([128, KT, 512], BF16, tag="xs", name="x_t")
            nc.sync.dma_start(
                x_t[:], (xq_r, xk_r, xv_r)[src_i][:, :, bass.ts(nch, 512)])
            return x_t

        def v_proj_compute(x_t, nch):
            # generator: one st-subtile (8 matmuls + evict) per step
            for stl in range(4):
                st = nch * 4 + stl
                ps = pe_pool.tile([128, 1024], F32, tag="pe", name="ps_v")
                for kt in range(KT):
                    nc.tensor.matmul(
                        ps[:, 0:512], x_t[:, kt, bass.ts(stl, 128)],
                        wv_sb[:, kt, :], start=(kt == 0), stop=(kt == KT - 1))
                nc.vector.tensor_tensor(
                    V_sb[:, st, :, 0:HD],
                    ps[:, 0:512].rearrange("p (h d) -> p h d", d=HD),
                    bv_bc.rearrange("p (h d) -> p h d", d=HD),
                    mybir.AluOpType.add)
                if stl < 3:
                    yield

        def qk_proj_compute(x_t, ti, nch, ms=range(MT)):
            # generator: selected m-tiles for one 512-seq x chunk (the chunk
            # is DMA'd once and shared across items); one m-tile (8 matmuls
            # + evict) per step
            w_sb = (wq_sb, wk_sb)[ti]
            b_sb = (bq_sb, bk_sb)[ti]
            dst = (QT_sb, KT_sb)[ti]
            for i, m in enumerate(ms):
                if i:
                    yield
                ps = pe_pool.tile([128, 1024], F32, tag="pe", name="ps_qk")
                for kt in range(KT):
                    nc.tensor.matmul(
                        ps[:, 0:512], w_sb[:, kt, bass.ts(m, 128)],
                        x_t[:, kt, :], start=(kt == 0), stop=(kt == KT - 1))
                nc.vector.tensor_scalar_add(
                    dst[:, m, bass.ts(nch, 512)], ps[:, 0:512],
                    b_sb[:, m:m + 1])

        def outproj_group(qt):
            # generator: one 512-wide embed half (4 matmuls + evict) per step
            ob = outp.tile([128, E], F32, tag="ob", name="ob")
            for ec in range(2):
                ps = pe_pool.tile([128, 1024], F32, tag="pe", name="ps_o")
                for mq in range(MT):
                    nc.tensor.matmul(
                        ps[:, 0:512], oT_sb[:, mq, bass.ts(qt, 128)],
                        wo_sb[:, mq, bass.ts(ec, 512)],
                        start=(mq == 0), stop=(mq == MT - 1))
                nc.vector.tensor_copy(ob[:, bass.ts(ec, 512)], ps[:, 0:512])
                if ec == 0:
                    yield
            nc.sync.dma_start(out_d[bass.ts(qt, 128), :], ob[:])

        def wo_dma(_t):
            nc.sync.dma_start(
                wo_sb[:], wo_d.rearrange("(mt p) e -> p mt e", p=128))
            return
            yield

        # ---- weave scheduler ----
        # One generator step per attention kt (two while the early-block
        # hard deps, items < N_FAST, are pending).  Items are m-quarter
        # granular so only what a block actually needs runs early; x chunks
        # are DMA'd once and shared across the items of that chunk via
        # x_tiles.  earliest_block (qb*MT+m) defers Q-proj and out-proj
        # into scalar-paced windows.
        # item: (chunk_key | None, factory, earliest_block)
        x_tiles = {}

        def chunk(src_i, nch):
            key = (src_i, nch)
            if key not in x_tiles:
                x_tiles[key] = x_dma(src_i, nch)
            return x_tiles[key]

        M123 = (1, 2, 3)
        items = [
            ((2, 1), lambda t: v_proj_compute(t, 1), 0),             # V st 4-7
            ((1, 1), lambda t: qk_proj_compute(t, 1, 1, (0,)), 0),   # K1 m0
            ((1, 0), lambda t: qk_proj_compute(t, 1, 0, M123), 0),   # K0 m1-3
            ((0, 0), lambda t: qk_proj_compute(t, 0, 0, M123), 0),   # Q0 m1-3
            ((2, 2), lambda t: v_proj_compute(t, 2), 0),             # V st 8-11
            ((1, 2), lambda t: qk_proj_compute(t, 1, 2, (0,)), 0),   # K2 m0
            ((2, 3), lambda t: v_proj_compute(t, 3), 0),             # V st 12-15
            ((1, 3), lambda t: qk_proj_compute(t, 1, 3, (0,)), 0),   # K3 m0
            ((1, 1), lambda t: qk_proj_compute(t, 1, 1, M123), 1),   # K1 m1-3
            ((1, 2), lambda t: qk_proj_compute(t, 1, 2, M123), 1),   # K2 m1-3
            ((1, 3), lambda t: qk_proj_compute(t, 1, 3, M123), 1),   # K3 m1-3
            ((0, 1), lambda t: qk_proj_compute(t, 0, 1), 2),         # Q1
            (None, wo_dma, 2),
            ((0, 2), lambda t: qk_proj_compute(t, 0, 2), 3),         # Q2
            ((0, 3), lambda t: qk_proj_compute(t, 0, 3), 4),         # Q3
        ]
        N_FAST = 8
        items += [(None, lambda _t, qt=qt: outproj_group(qt),
                   min((qt // 4 + 1) * MT + 1 + qt % 4, 15))
                  for qt in range(12)]

        wv_state = {"pi": 0, "gi": 0, "tiles": {}, "gen": None}

        def weave_prefetch():
            pi = wv_state["pi"]
            if pi < len(items) and pi - wv_state["gi"] < 3:
                key, _, _ = items[pi]
                wv_state["tiles"][pi] = chunk(*key) if key is not None else None
                wv_state["pi"] = pi + 1

        def weave_step(block):
            st = wv_state
            if (st["gen"] is None and st["gi"] < st["pi"]
                    and items[st["gi"]][2] <= block):
                st["gen"] = items[st["gi"]][1](st["tiles"].pop(st["gi"]))
            if st["gen"] is not None:
                try:
                    next(st["gen"])
                except StopIteration:
                    st["gen"] = None
                    st["gi"] += 1
            weave_prefetch()

        def run_whole(gen):
            for _ in gen:
                pass

        # ---- prologue: only what attention block 0 needs up front —
        # chunk-0 K/Q m=0 quarters and V st 0..3; everything else weaves.
        # DMAs issued in first-use order. ----
        xk0 = chunk(1, 0)
        nc.sync.dma_start(wq_sb[:], wq_d.rearrange("(kt p) m -> p kt m", p=128))
        nc.sync.dma_start(bq_sb[:], bq_d)
        xq0 = chunk(0, 0)
        nc.sync.dma_start(wv_sb[:], wv_d.rearrange("(kt p) m -> p kt m", p=128))
        nc.sync.dma_start(bv_row[:], bv_d)
        nc.gpsimd.partition_broadcast(bv_bc[:], bv_row[:])
        xv0 = chunk(2, 0)
        xv1 = chunk(2, 1)
        run_whole(qk_proj_compute(xk0, 1, 0, (0,)))
        run_whole(qk_proj_compute(xq0, 0, 0, (0,)))
        run_whole(v_proj_compute(xv0, 0))
        weave_prefetch()          # first woven item's chunk is xv1 (queued)

        # ---- attention, with next m's Q/K projections woven in ----
        # Heads 2m / 2m+1 live on partitions 0:64 / 64:128 of QT/KT tile m,
        # so their energy matmuls auto-derive PE tile_position (0,0)/(64,0)
        # and run concurrently (row tiling) when issued back to back.  One
        # pe tile holds both heads' energies for a 512-wide q block, so one
        # exp activation covers the pair.
        for qb in range(4):                      # q blocks of 512
            for m in range(MT):
                q0 = qb * 512
                block = qb * MT + m
                po0 = po_pool.tile([HD + 1, 512], F32, tag="po", name="po0")
                po1 = po_pool.tile([HD + 1, 512], F32, tag="po", name="po1")
                pending = None
                for kt in range(ST):
                    pe = pe_pool.tile([128, 1024], F32, tag="pe")
                    for hs in range(2):
                        psl = slice(64 * hs, 64 * hs + 64)
                        nc.tensor.matmul(
                            pe[:, bass.ts(hs, 512)],
                            KT_sb[psl, m, bass.ts(kt, 128)],
                            QT_sb[psl, m, bass.ds(q0, 512)],
                            start=True, stop=True)
                    at = atp.tile([128, 1024], BF16, tag="at")
                    nc.scalar.activation(at[:], pe[:], EXP)
                    if pending is not None:
                        pkt, pat = pending
                        for hs, po in ((0, po0), (1, po1)):
                            nc.tensor.matmul(
                                po[:, :], V_sb[:, pkt, 2 * m + hs, :],
                                pat[:, bass.ts(hs, 512)],
                                start=(pkt == 0), stop=(pkt == ST - 1))
                    pending = (kt, at)
                    weave_step(block)
                    if wv_state["gi"] < N_FAST:
                        weave_step(block)   # block-0 hard deps: drain fast
                pkt, pat = pending
                for hs, po in ((0, po0), (1, po1)):
                    nc.tensor.matmul(
                        po[:, :], V_sb[:, pkt, 2 * m + hs, :],
                        pat[:, bass.ts(hs, 512)],
                        start=(pkt == 0), stop=(pkt == ST - 1))

                # ---- normalize + evict ----
                # 1/sums straight out of PSUM row 64 into partition 0 of a
                # fresh SBUF tile (partition_broadcast ucode reads the
                # physical first partition of its input).
                for hs, po in ((0, po0), (1, po1)):
                    s_sb = smp.tile([1, 512], F32, tag="s")
                    nc.vector.tensor_copy(s_sb[0:1, :], po[HD:HD + 1, :])
                    nc.vector.reciprocal_approx_fast(s_sb[0:1, :], s_sb[0:1, :])
                    bc = smp.tile([HD, 512], F32, tag="bc")
                    nc.gpsimd.partition_broadcast(bc[:], s_sb[0:1, :])
                    nc.vector.tensor_tensor(
                        oT_sb[64 * hs:64 * hs + HD, m, bass.ds(q0, 512)],
                        po[0:HD, :], bc[:], mybir.AluOpType.mult)

        if dump:
            d_qt = nc.dram_tensor("d_qt", [128, MT, S], BF16, kind="ExternalOutput").ap()
            d_kt = nc.dram_tensor("d_kt", [128, MT, S], BF16, kind="ExternalOutput").ap()
            d_v = nc.dram_tensor("d_v", [128, ST, NHL, HD + 1], BF16, kind="ExternalOutput").ap()
            d_ot = nc.dram_tensor("d_ot", [128, MT, S], BF16, kind="ExternalOutput").ap()
            nc.sync.dma_start(d_qt, QT_sb[:])
            nc.sync.dma_start(d_kt, KT_sb[:])
            nc.sync.dma_start(d_v, V_sb[:])
            nc.sync.dma_start(d_ot, oT_sb[:])

        # ---- tail: remaining woven groups + final out-projection ----
        while wv_state["gi"] < len(items):
            weave_step(10 ** 9)
        for qt in range(12, ST):
            run_whole(outproj_group(qt))


_CACHED = {}


def _get_bass():
    if "nc" not in _CACHED:
        _CACHED["nc"] = _build_bass()
    return _CACHED["nc"]


def _prep_core_inputs(c, query, key, value, Wq, bq, Wk, bk, Wv, bv, Wo):
    b, half = c // 2, c % 2
    sl = slice(DLOC * half, DLOC * half + DLOC)
    bq_sl = (bq[sl] * 0.125).astype(np.float32).reshape(MT, 128).T.copy()
    bk_sl = bk[sl].astype(np.float32).reshape(MT, 128).T.copy()
    return {
        "xqT": np.ascontiguousarray(query[b].T).astype(NPBF),
        "xkT": np.ascontiguousarray(key[b].T).astype(NPBF),
        "xvT": np.ascontiguousarray(value[b].T).astype(NPBF),
        "wq": np.ascontiguousarray(Wq[sl, :].T * 0.125).astype(NPBF),
        "wk": np.ascontiguousarray(Wk[sl, :].T).astype(NPBF),
        "wv": np.ascontiguousarray(Wv[sl, :].T).astype(NPBF),
        "wo": np.ascontiguousarray(Wo[:, sl].T).astype(NPBF),
        "bq": np.ascontiguousarray(bq_sl),
        "bk": np.ascontiguousarray(bk_sl),
        "bv": bv[sl].astype(np.float32).reshape(1, DLOC).copy(),
    }


def kernel(query, key, value, Wq, bq, Wk, bk, Wv, bv, Wo, bo,
           trace=False, **run_kwargs):
    query = np.asarray(query, np.float32)
    key = np.asarray(key, np.float32)
    value = np.asarray(value, np.float32)
    Wq, Wk, Wv, Wo = (np.asarray(w, np.float32) for w in (Wq, Wk, Wv, Wo))
    bq, bk, bv, bo = (np.asarray(x, np.float32) for x in (bq, bk, bv, bo))

    nc = _get_bass()
    in_maps = [_prep_core_inputs(c, query, key, value, Wq, bq, Wk, bk, Wv, bv, Wo)
               for c in range(8)]
    res = run_bass_kernel_spmd(nc, in_maps, core_ids=list(range(8)),
                               trace=trace, **run_kwargs)
    _CACHED["last_result"] = res

    B = query.shape[0]
    out = np.empty((B, S, E), np.float32)
    for b in range(B):
        out[b] = res.results[2 * b]["out"] + res.results[2 * b + 1]["out"] + bo
    return out



# revision 36
# speedup vs baseline: 1.0065x; 1.0065x over previous
"""Multi-head attention (B=4, S=2048, E=1024, 16 heads x 64) on 8 Trainium2 cores.

Sharding: core c = 2*b + half handles batch b and heads [8*half, 8*half+8)
(embed slice [512*half, 512*half+512)).  Each core computes its Q/K/V
projections, 8 heads of attention, and a row-parallel out-projection partial
(2048, 1024).  Host unshard: out[b] = partial[2b] + partial[2b+1] + bo.

Per-core device kernel (bf16 matmuls, fp32 accumulation):
  - QT/KT in [d_local, seq] layout (d on partitions) so energy^T = K @ Q^T
    comes out as [k_seq, q_seq] with softmax reductions computable by matmul.
  - softmax without max subtraction (energies are ~N(0,1); exp never overflows)
    with 1/sqrt(64) folded into Wq on the host.
  - exp on the scalar engine straight out of PSUM, bf16 output.
  - V carries an appended ones column so the attn@V matmul (M=65) yields the
    softmax denominator for free in PSUM row 64.
  - normalization: reciprocal of the sums row, gpsimd partition_broadcast,
    multiply-on-evict; odd heads are repacked to partitions 64:128 via
    SBUF->SBUF DMA so the out-projection contracts K=128 per matmul.
"""

import numpy as np
import ml_dtypes

import concourse.bass as bass
import concourse.mybir as mybir
import concourse.tile as tile
import concourse.bacc as bacc
from concourse.bass_utils import run_bass_kernel_spmd

BF16 = mybir.dt.bfloat16
F32 = mybir.dt.float32
NPBF = ml_dtypes.bfloat16

S = 2048          # sequence length
E = 1024          # embed dim
DLOC = 512        # per-core embed slice (8 heads x 64)
HD = 64           # head dim
NHL = 8           # heads per core
KT = E // 128     # 8 contraction tiles for projections
MT = DLOC // 128  # 4 m-tiles of d_local
ST = S // 128     # 16 seq tiles
NCH = S // 512    # 4 seq chunks of 512
EXP = mybir.ActivationFunctionType.Exp


def _build_bass(dump=False):
    nc = bacc.Bacc("TRN2", target_bir_lowering=False, debug=False)

    xqT = nc.dram_tensor("xqT", [E, S], BF16, kind="ExternalInput").ap()
    xkT = nc.dram_tensor("xkT", [E, S], BF16, kind="ExternalInput").ap()
    xvT = nc.dram_tensor("xvT", [E, S], BF16, kind="ExternalInput").ap()
    wq_d = nc.dram_tensor("wq", [E, DLOC], BF16, kind="ExternalInput").ap()
    wk_d = nc.dram_tensor("wk", [E, DLOC], BF16, kind="ExternalInput").ap()
    wv_d = nc.dram_tensor("wv", [E, DLOC], BF16, kind="ExternalInput").ap()
    wo_d = nc.dram_tensor("wo", [DLOC, E], BF16, kind="ExternalInput").ap()
    bq_d = nc.dram_tensor("bq", [128, MT], F32, kind="ExternalInput").ap()
    bk_d = nc.dram_tensor("bk", [128, MT], F32, kind="ExternalInput").ap()
    bv_d = nc.dram_tensor("bv", [1, DLOC], F32, kind="ExternalInput").ap()
    out_d = nc.dram_tensor("out", [S, E], F32, kind="ExternalOutput").ap()

    xq_r = xqT.rearrange("(kt p) s -> p kt s", p=128)
    xk_r = xkT.rearrange("(kt p) s -> p kt s", p=128)
    xv_r = xvT.rearrange("(kt p) s -> p kt s", p=128)

    with tile.TileContext(nc) as tc:
        _kernel_body(tc, nc, xq_r, xk_r, xv_r, wq_d, wk_d, wv_d, wo_d,
                     bq_d, bk_d, bv_d, out_d, dump=dump)
    nc.compile()
    return nc


def _kernel_body(tc, nc, xq_r, xk_r, xv_r, wq_d, wk_d, wv_d, wo_d,
                 bq_d, bk_d, bv_d, out_d, dump=False):
    from contextlib import ExitStack

    with ExitStack() as ctx:
        wpool = ctx.enter_context(tc.tile_pool(name="weights", bufs=1))
        xpool = ctx.enter_context(tc.tile_pool(name="xstream", bufs=8))
        qkv = ctx.enter_context(tc.tile_pool(name="qkv", bufs=1))
        atp = ctx.enter_context(tc.tile_pool(name="attnt", bufs=4))
        smp = ctx.enter_context(tc.tile_pool(name="small", bufs=2))
        outp = ctx.enter_context(tc.tile_pool(name="outstage", bufs=3))

        # ---- weights / biases to SBUF ----
        wq_sb = wpool.tile([128, KT, DLOC], BF16)
        wk_sb = wpool.tile([128, KT, DLOC], BF16)
        wv_sb = wpool.tile([128, KT, DLOC], BF16)
        wo_sb = wpool.tile([128, MT, E], BF16)
        bq_sb = wpool.tile([128, MT], F32)
        bk_sb = wpool.tile([128, MT], F32)
        bv_row = wpool.tile([1, DLOC], F32)
        bv_bc = wpool.tile([128, DLOC], F32)
        # (weight DMAs are interleaved with the prologue's x-chunk DMAs below
        # so the first K-projection isn't stuck behind wo/wv in the queue)
        nc.sync.dma_start(wk_sb[:], wk_d.rearrange("(kt p) m -> p kt m", p=128))
        nc.sync.dma_start(bk_sb[:], bk_d)

        # ---- persistent per-core tensors ----
        QT_sb = qkv.tile([128, MT, S], BF16)        # [d_loc, seq]
        KT_sb = qkv.tile([128, MT, S], BF16)
        V_sb = qkv.tile([128, ST, NHL, HD + 1], BF16)  # ones col at 64
        oT_sb = qkv.tile([128, MT, S], BF16)        # attn out^T (lhsT of outproj)

        nc.vector.memset(V_sb[:, :, :, HD:HD + 1], 1.0)

        # One PSUM layout for the whole kernel: 2x [128,1024] energy slots
        # (also used by proj/outproj psums) + 2x [65,1024] attn-out slots.
        pe_pool = ctx.enter_context(tc.tile_pool(name="psum_e", bufs=2, space="PSUM"))
        po_pool = ctx.enter_context(tc.tile_pool(name="psum_o", bufs=4, space="PSUM"))

        def x_dma(src_i, nch):
            x_t = xpool.tile([128, KT, 512], BF16, tag="xs", name="x_t")
            nc.sync.dma_start(
                x_t[:], (xq_r, xk_r, xv_r)[src_i][:, :, bass.ts(nch, 512)])
            return x_t

        def v_proj_compute(x_t, nch):
            # generator: one st-subtile (8 matmuls + evict) per step
            for stl in range(4):
                st = nch * 4 + stl
                ps = pe_pool.tile([128, 1024], F32, tag="pe", name="ps_v")
                for kt in range(KT):
                    nc.tensor.matmul(
                        ps[:, 0:512], x_t[:, kt, bass.ts(stl, 128)],
                        wv_sb[:, kt, :], start=(kt == 0), stop=(kt == KT - 1))
                nc.vector.tensor_tensor(
                    V_sb[:, st, :, 0:HD],
                    ps[:, 0:512].rearrange("p (h d) -> p h d", d=HD),
                    bv_bc.rearrange("p (h d) -> p h d", d=HD),
                    mybir.AluOpType.add)
                if stl < 3:
                    yield

        def qk_proj_compute(x_t, ti, nch, ms=range(MT)):
            # generator: selected m-tiles for one 512-seq x chunk (the chunk
            # is DMA'd once and shared across items); one m-tile (8 matmuls
            # + evict) per step
            w_sb = (wq_sb, wk_sb)[ti]
            b_sb = (bq_sb, bk_sb)[ti]
            dst = (QT_sb, KT_sb)[ti]
            for i, m in enumerate(ms):
                if i:
                    yield
                ps = pe_pool.tile([128, 1024], F32, tag="pe", name="ps_qk")
                for kt in range(KT):
                    nc.tensor.matmul(
                        ps[:, 0:512], w_sb[:, kt, bass.ts(m, 128)],
                        x_t[:, kt, :], start=(kt == 0), stop=(kt == KT - 1))
                nc.vector.tensor_scalar_add(
                    dst[:, m, bass.ts(nch, 512)], ps[:, 0:512],
                    b_sb[:, m:m + 1])

        def outproj_group(qt):
            # generator: one 512-wide embed half (4 matmuls + evict) per step
            ob = outp.tile([128, E], F32, tag="ob", name="ob")
            for ec in range(2):
                ps = pe_pool.tile([128, 1024], F32, tag="pe", name="ps_o")
                for mq in range(MT):
                    nc.tensor.matmul(
                        ps[:, 0:512], oT_sb[:, mq, bass.ts(qt, 128)],
                        wo_sb[:, mq, bass.ts(ec, 512)],
                        start=(mq == 0), stop=(mq == MT - 1))
                nc.vector.tensor_copy(ob[:, bass.ts(ec, 512)], ps[:, 0:512])
                if ec == 0:
                    yield
            nc.sync.dma_start(out_d[bass.ts(qt, 128), :], ob[:])

        def wo_dma(_t):
            nc.sync.dma_start(
                wo_sb[:], wo_d.rearrange("(mt p) e -> p mt e", p=128))
            return
            yield

        # ---- weave scheduler ----
        # One generator step per attention kt (two while the early-block
        # hard deps, items < N_FAST, are pending).  Items are m-quarter
        # granular so only what a block actually needs runs early; x chunks
        # are DMA'd once and shared across the items of that chunk via
        # x_tiles.  earliest_block (qb*MT+m) defers Q-proj and out-proj
        # into scalar-paced windows.
        # item: (chunk_key | None, factory, earliest_block)
        x_tiles = {}
        x_allocs = []

        def chunk(src_i, nch):
            key = (src_i, nch)
            if key not in x_tiles:
                x_allocs.append(key)
                x_tiles[key] = x_dma(src_i, nch)
            return x_tiles[key]

        items = [
            ((2, 0), lambda t: v_proj_compute(t, 0), 0),               # V st0-3
            ((1, 0), lambda t: qk_proj_compute(t, 1, 0, (1,)), 0),     # K0 m1
            ((1, 0), lambda t: qk_proj_compute(t, 1, 0, (2, 3)), 0),   # K0 m23
            ((1, 1), lambda t: qk_proj_compute(t, 1, 1, (0, 1)), 0),   # K1 m01
            ((2, 1), lambda t: v_proj_compute(t, 1), 0),               # V st4-7
            ((1, 2), lambda t: qk_proj_compute(t, 1, 2, (0, 1)), 0),   # K2 m01
            ((2, 2), lambda t: v_proj_compute(t, 2), 0),               # V st8-11
            ((1, 3), lambda t: qk_proj_compute(t, 1, 3, (0, 1)), 0),   # K3 m01
            ((2, 3), lambda t: v_proj_compute(t, 3), 0),               # V st12-15
            ((0, 0), lambda t: qk_proj_compute(t, 0, 0, (1,)), 0),     # Q0 m1
            ((1, 1), lambda t: qk_proj_compute(t, 1, 1, (2, 3)), 1),   # K1 m23
            ((1, 2), lambda t: qk_proj_compute(t, 1, 2, (2, 3)), 1),   # K2 m23
            ((0, 0), lambda t: qk_proj_compute(t, 0, 0, (2,)), 1),     # Q0 m2
            ((1, 3), lambda t: qk_proj_compute(t, 1, 3, (2, 3)), 2),   # K3 m23
            ((0, 0), lambda t: qk_proj_compute(t, 0, 0, (3,)), 2),     # Q0 m3
            ((0, 1), lambda t: qk_proj_compute(t, 0, 1), 3),           # Q1
            (None, wo_dma, 3),
            ((0, 2), lambda t: qk_proj_compute(t, 0, 2), 4),           # Q2
            ((0, 3), lambda t: qk_proj_compute(t, 0, 3), 5),           # Q3
        ]
        N_FAST = 10     # block-0 hard deps: 2 steps/kt until done
        items += [(None, lambda _t, qt=qt: outproj_group(qt),
                   min((qt // 4 + 1) * MT + 1 + qt % 4, 15))
                  for qt in range(12)]
        last_ref = {}
        for idx, (key, _, _) in enumerate(items):
            if key is not None:
                last_ref[key] = idx

        XBUFS = 8
        wv_state = {"pi": 0, "gi": 0, "tiles": {}, "gen": None}

        def weave_prefetch():
            pi = wv_state["pi"]
            if pi >= len(items) or pi - wv_state["gi"] >= 3:
                return
            key = items[pi][0]
            if key is not None and key not in x_tiles:
                # the xpool ring recycles the (n-XBUFS)-th allocation's
                # buffer; don't DMA over a chunk a pending item still reads
                n = len(x_allocs)
                if n >= XBUFS:
                    victim = x_allocs[n - XBUFS]
                    if last_ref.get(victim, -1) >= wv_state["gi"]:
                        return
            wv_state["tiles"][pi] = chunk(*key) if key is not None else None
            wv_state["pi"] = pi + 1

        def weave_step(block):
            st = wv_state
            if (st["gen"] is None and st["gi"] < st["pi"]
                    and items[st["gi"]][2] <= block):
                st["gen"] = items[st["gi"]][1](st["tiles"].pop(st["gi"]))
            if st["gen"] is not None:
                try:
                    next(st["gen"])
                except StopIteration:
                    st["gen"] = None
                    st["gi"] += 1
            weave_prefetch()

        def run_whole(gen):
            for _ in gen:
                pass

        # ---- prologue: only what attention block 0 needs up front —
        # chunk-0 K/Q m=0 quarters; V and everything else weaves.
        # DMAs issued in first-use order. ----
        xk0 = chunk(1, 0)
        nc.sync.dma_start(wq_sb[:], wq_d.rearrange("(kt p) m -> p kt m", p=128))
        nc.sync.dma_start(bq_sb[:], bq_d)
        xq0 = chunk(0, 0)
        nc.sync.dma_start(wv_sb[:], wv_d.rearrange("(kt p) m -> p kt m", p=128))
        nc.sync.dma_start(bv_row[:], bv_d)
        # warm the exp table set while DMAs stream (one-time ~2.7us load)
        warm_in = smp.tile([1, 8], F32, tag="warm_i", name="warm_i")
        warm = smp.tile([1, 8], BF16, tag="warm_o", name="warm_o")
        nc.vector.memset(warm_in[:], 0.0)
        nc.scalar.activation(warm[:], warm_in[:], EXP)
        nc.gpsimd.partition_broadcast(bv_bc[:], bv_row[:])
        xv0 = chunk(2, 0)
        xv1 = chunk(2, 1)
        run_whole(qk_proj_compute(xk0, 1, 0, (0,)))
        run_whole(qk_proj_compute(xq0, 0, 0, (0,)))
        weave_prefetch()          # first woven item's chunk is xv0 (queued)

        # ---- attention, with next m's Q/K projections woven in ----
        # Heads 2m / 2m+1 live on partitions 0:64 / 64:128 of QT/KT tile m,
        # so their energy matmuls auto-derive PE tile_position (0,0)/(64,0)
        # and run concurrently (row tiling) when issued back to back.  One
        # pe tile holds both heads' energies for a 512-wide q block, so one
        # exp activation covers the pair.
        for qb in range(4):                      # q blocks of 512
            for m in range(MT):
                q0 = qb * 512
                block = qb * MT + m
                po0 = po_pool.tile([HD + 1, 512], F32, tag="po", name="po0")
                po1 = po_pool.tile([HD + 1, 512], F32, tag="po", name="po1")
                pending = None
                for kt in range(ST):
                    pe = pe_pool.tile([128, 1024], F32, tag="pe")
                    for hs in range(2):
                        psl = slice(64 * hs, 64 * hs + 64)
                        nc.tensor.matmul(
                            pe[:, bass.ts(hs, 512)],
                            KT_sb[psl, m, bass.ts(kt, 128)],
                            QT_sb[psl, m, bass.ds(q0, 512)],
                            start=True, stop=True)
                    at = atp.tile([128, 1024], BF16, tag="at")
                    nc.scalar.activation(at[:], pe[:], EXP)
                    if pending is not None:
                        pkt, pat = pending
                        for hs, po in ((0, po0), (1, po1)):
                            nc.tensor.matmul(
                                po[:, :], V_sb[:, pkt, 2 * m + hs, :],
                                pat[:, bass.ts(hs, 512)],
                                start=(pkt == 0), stop=(pkt == ST - 1))
                    pending = (kt, at)
                    weave_step(block)
                    if wv_state["gi"] < N_FAST:
                        weave_step(block)   # block-0 hard deps: 2 steps/kt
                pkt, pat = pending
                for hs, po in ((0, po0), (1, po1)):
                    nc.tensor.matmul(
                        po[:, :], V_sb[:, pkt, 2 * m + hs, :],
                        pat[:, bass.ts(hs, 512)],
                        start=(pkt == 0), stop=(pkt == ST - 1))

                # ---- normalize + evict ----
                # 1/sums straight out of PSUM row 64 into partition 0 of a
                # fresh SBUF tile (partition_broadcast ucode reads the
                # physical first partition of its input).
                for hs, po in ((0, po0), (1, po1)):
                    s_sb = smp.tile([1, 512], F32, tag="s")
                    nc.vector.tensor_copy(s_sb[0:1, :], po[HD:HD + 1, :])
                    nc.vector.reciprocal_approx_fast(s_sb[0:1, :], s_sb[0:1, :])
                    bc = smp.tile([HD, 512], F32, tag="bc")
                    nc.gpsimd.partition_broadcast(bc[:], s_sb[0:1, :])
                    nc.vector.tensor_tensor(
                        oT_sb[64 * hs:64 * hs + HD, m, bass.ds(q0, 512)],
                        po[0:HD, :], bc[:], mybir.AluOpType.mult)

        if dump:
            d_qt = nc.dram_tensor("d_qt", [128, MT, S], BF16, kind="ExternalOutput").ap()
            d_kt = nc.dram_tensor("d_kt", [128, MT, S], BF16, kind="ExternalOutput").ap()
            d_v = nc.dram_tensor("d_v", [128, ST, NHL, HD + 1], BF16, kind="ExternalOutput").ap()
            d_ot = nc.dram_tensor("d_ot", [128, MT, S], BF16, kind="ExternalOutput").ap()
            nc.sync.dma_start(d_qt, QT_sb[:])
            nc.sync.dma_start(d_kt, KT_sb[:])
            nc.sync.dma_start(d_v, V_sb[:])
            nc.sync.dma_start(d_ot, oT_sb[:])

        # ---- tail: remaining woven groups + final out-projection ----
        while wv_state["gi"] < len(items):
            weave_step(10 ** 9)
        for qt in range(12, ST):
            run_whole(outproj_group(qt))


_CACHED = {}


def _get_bass():
    if "nc" not in _CACHED:
        _CACHED["nc"] = _build_bass()
    return _CACHED["nc"]


def _prep_core_inputs(c, query, key, value, Wq, bq, Wk, bk, Wv, bv, Wo):
    b, half = c // 2, c % 2
    sl = slice(DLOC * half, DLOC * half + DLOC)
    bq_sl = (bq[sl] * 0.125).astype(np.float32).reshape(MT, 128).T.copy()
    bk_sl = bk[sl].astype(np.float32).reshape(MT, 128).T.copy()
    return {
        "xqT": np.ascontiguousarray(query[b].T).astype(NPBF),
        "xkT": np.ascontiguousarray(key[b].T).astype(NPBF),
        "xvT": np.ascontiguousarray(value[b].T).astype(NPBF),
        "wq": np.ascontiguousarray(Wq[sl, :].T * 0.125).astype(NPBF),
        "wk": np.ascontiguousarray(Wk[sl, :].T).astype(NPBF),
        "wv": np.ascontiguousarray(Wv[sl, :].T).astype(NPBF),
        "wo": np.ascontiguousarray(Wo[:, sl].T).astype(NPBF),
        "bq": np.ascontiguousarray(bq_sl),
        "bk": np.ascontiguousarray(bk_sl),
        "bv": bv[sl].astype(np.float32).reshape(1, DLOC).copy(),
    }


def kernel(query, key, value, Wq, bq, Wk, bk, Wv, bv, Wo, bo,
           trace=False, **run_kwargs):
    query = np.asarray(query, np.float32)
    key = np.asarray(key, np.float32)
    value = np.asarray(value, np.float32)
    Wq, Wk, Wv, Wo = (np.asarray(w, np.float32) for w in (Wq, Wk, Wv, Wo))
    bq, bk, bv, bo = (np.asarray(x, np.float32) for x in (bq, bk, bv, bo))

    nc = _get_bass()
    in_maps = [_prep_core_inputs(c, query, key, value, Wq, bq, Wk, bk, Wv, bv, Wo)
               for c in range(8)]
    res = run_bass_kernel_spmd(nc, in_maps, core_ids=list(range(8)),
                               trace=trace, **run_kwargs)
    _CACHED["last_result"] = res

    B = query.shape[0]
    out = np.empty((B, S, E), np.float32)
    for b in range(B):
        out[b] = res.results[2 * b]["out"] + res.results[2 * b + 1]["out"] + bo
    return out



# revision 56
# speedup vs baseline: 1.0479x; 1.0411x over previous
"""Multi-head attention (B=4, S=2048, E=1024, 16 heads x 64) on 8 Trainium2 cores.

Sharding: core c = 2*b + half handles batch b and heads [8*half, 8*half+8)
(embed slice [512*half, 512*half+512)).  Each core computes its Q/K/V
projections, 8 heads of attention, and a row-parallel out-projection partial
(2048, 1024).  Host unshard: out[b] = partial[2b] + partial[2b+1] + bo.

Per-core device kernel (bf16 matmuls, fp32 accumulation):
  - QT/KT in [d_local, seq] layout (d on partitions) so energy^T = K @ Q^T
    comes out as [k_seq, q_seq] with softmax reductions computable by matmul.
  - softmax without max subtraction (energies are ~N(0,1); exp never overflows)
    with 1/sqrt(64) folded into Wq on the host.
  - exp on the scalar engine straight out of PSUM, bf16 output.
  - V carries an appended ones column so the attn@V matmul (M=65) yields the
    softmax denominator for free in PSUM row 64.
  - normalization: reciprocal of the sums row, gpsimd partition_broadcast,
    multiply-on-evict; odd heads are repacked to partitions 64:128 via
    SBUF->SBUF DMA so the out-projection contracts K=128 per matmul.
"""

import numpy as np
import ml_dtypes

import concourse.bass as bass
import concourse.mybir as mybir
import concourse.tile as tile
import concourse.bacc as bacc
from concourse.bass_utils import run_bass_kernel_spmd

BF16 = mybir.dt.bfloat16
F32 = mybir.dt.float32
NPBF = ml_dtypes.bfloat16

S = 2048          # sequence length
E = 1024          # embed dim
DLOC = 512        # per-core embed slice (8 heads x 64)
HD = 64           # head dim
NHL = 8           # heads per core
KT = E // 128     # 8 contraction tiles for projections
MT = DLOC // 128  # 4 m-tiles of d_local
ST = S // 128     # 16 seq tiles
NCH = S // 512    # 4 seq chunks of 512
EXP = mybir.ActivationFunctionType.Exp


def _build_bass(dump=False):
    nc = bacc.Bacc("TRN2", target_bir_lowering=False, debug=False)

    xqT = nc.dram_tensor("xqT", [E, S], BF16, kind="ExternalInput").ap()
    xkT = nc.dram_tensor("xkT", [E, S], BF16, kind="ExternalInput").ap()
    xvT = nc.dram_tensor("xvT", [E, S], BF16, kind="ExternalInput").ap()
    wq_d = nc.dram_tensor("wq", [E, DLOC], BF16, kind="ExternalInput").ap()
    wk_d = nc.dram_tensor("wk", [E, DLOC], BF16, kind="ExternalInput").ap()
    wv_d = nc.dram_tensor("wv", [E, DLOC], BF16, kind="ExternalInput").ap()
    wo_d = nc.dram_tensor("wo", [DLOC, E], BF16, kind="ExternalInput").ap()
    bq_d = nc.dram_tensor("bq", [128, MT], F32, kind="ExternalInput").ap()
    bk_d = nc.dram_tensor("bk", [128, MT], F32, kind="ExternalInput").ap()
    bv_d = nc.dram_tensor("bv", [1, DLOC], F32, kind="ExternalInput").ap()
    out_d = nc.dram_tensor("out", [S, E], F32, kind="ExternalOutput").ap()

    xq_r = xqT.rearrange("(kt p) s -> p kt s", p=128)
    xk_r = xkT.rearrange("(kt p) s -> p kt s", p=128)
    xv_r = xvT.rearrange("(kt p) s -> p kt s", p=128)

    with tile.TileContext(nc) as tc:
        _kernel_body(tc, nc, xq_r, xk_r, xv_r, wq_d, wk_d, wv_d, wo_d,
                     bq_d, bk_d, bv_d, out_d, dump=dump)
    nc.compile()
    return nc


def _kernel_body(tc, nc, xq_r, xk_r, xv_r, wq_d, wk_d, wv_d, wo_d,
                 bq_d, bk_d, bv_d, out_d, dump=False):
    from contextlib import ExitStack

    with ExitStack() as ctx:
        wpool = ctx.enter_context(tc.tile_pool(name="weights", bufs=1))
        xpool = ctx.enter_context(tc.tile_pool(name="xstream", bufs=8))
        qkv = ctx.enter_context(tc.tile_pool(name="qkv", bufs=1))
        atp = ctx.enter_context(tc.tile_pool(name="attnt", bufs=4))
        smp = ctx.enter_context(tc.tile_pool(name="small", bufs=2))
        outp = ctx.enter_context(tc.tile_pool(name="outstage", bufs=3))

        # ---- weights / biases to SBUF ----
        wq_sb = wpool.tile([128, KT, DLOC], BF16)
        wk_sb = wpool.tile([128, KT, DLOC], BF16)
        wv_sb = wpool.tile([128, KT, DLOC], BF16)
        wo_sb = wpool.tile([128, MT, E], BF16)
        bq_sb = wpool.tile([128, MT], F32)
        bk_sb = wpool.tile([128, MT], F32)
        bv_row = wpool.tile([1, DLOC], F32)
        bv_bc = wpool.tile([128, DLOC], F32)
        # (weight DMAs are interleaved with the prologue's x-chunk DMAs below
        # so the first K-projection isn't stuck behind wo/wv in the queue)
        nc.sync.dma_start(wk_sb[:], wk_d.rearrange("(kt p) m -> p kt m", p=128))
        nc.sync.dma_start(bk_sb[:], bk_d)

        # ---- persistent per-core tensors ----
        QT_sb = qkv.tile([128, MT, S], BF16)        # [d_loc, seq]
        KT_sb = qkv.tile([128, MT, S], BF16)
        V_sb = qkv.tile([128, ST, NHL, HD + 1], BF16)  # ones col at 64
        oT_sb = qkv.tile([128, MT, S], BF16)        # attn out^T (lhsT of outproj)

        nc.vector.memset(V_sb[:, :, :, HD:HD + 1], 1.0)

        # One PSUM layout for the whole kernel: 2x [128,1024] energy slots
        # (also used by proj/outproj psums) + 2x [65,1024] attn-out slots.
        pe_pool = ctx.enter_context(tc.tile_pool(name="psum_e", bufs=2, space="PSUM"))
        po_pool = ctx.enter_context(tc.tile_pool(name="psum_o", bufs=4, space="PSUM"))

        def x_dma(src_i, nch):
            x_t = xpool.tile([128, KT, 512], BF16, tag="xs", name="x_t")
            nc.sync.dma_start(
                x_t[:], (xq_r, xk_r, xv_r)[src_i][:, :, bass.ts(nch, 512)])
            return x_t

        def v_proj_compute(x_t, nch):
            # generator: one st-subtile (8 matmuls + evict) per step
            for stl in range(4):
                st = nch * 4 + stl
                if stl:
                    yield
                ps = pe_pool.tile([128, 1024], F32, tag="pe", name="ps_v")
                for kt in range(KT):
                    nc.tensor.matmul(
                        ps[:, 0:512], x_t[:, kt, bass.ts(stl, 128)],
                        wv_sb[:, kt, :], start=(kt == 0), stop=(kt == KT - 1))
                nc.vector.tensor_tensor(
                    V_sb[:, st, :, 0:HD],
                    ps[:, 0:512].rearrange("p (h d) -> p h d", d=HD),
                    bv_bc.rearrange("p (h d) -> p h d", d=HD),
                    mybir.AluOpType.add)

        def qk_proj_compute(x_t, ti, nch):
            # generator: all four m-tiles for one 512-seq x chunk (the chunk
            # is DMA'd once); one m-tile (8 matmuls + evict) per step, m=0
            # first since the attention consumes it soonest
            w_sb = (wq_sb, wk_sb)[ti]
            b_sb = (bq_sb, bk_sb)[ti]
            dst = (QT_sb, KT_sb)[ti]
            for m in range(MT):
                if m:
                    yield
                ps = pe_pool.tile([128, 1024], F32, tag="pe", name="ps_qk")
                for kt in range(KT):
                    nc.tensor.matmul(
                        ps[:, 0:512], w_sb[:, kt, bass.ts(m, 128)],
                        x_t[:, kt, :], start=(kt == 0), stop=(kt == KT - 1))
                nc.vector.tensor_scalar_add(
                    dst[:, m, bass.ts(nch, 512)], ps[:, 0:512],
                    b_sb[:, m:m + 1])

        def outproj_group(qt):
            # generator: one 512-wide embed half (4 matmuls + evict) per step
            ob = outp.tile([128, E], F32, tag="ob", name="ob")
            for ec in range(2):
                if ec:
                    yield
                ps = pe_pool.tile([128, 1024], F32, tag="pe", name="ps_o")
                for mq in range(MT):
                    nc.tensor.matmul(
                        ps[:, 0:512], oT_sb[:, mq, bass.ts(qt, 128)],
                        wo_sb[:, mq, bass.ts(ec, 512)],
                        start=(mq == 0), stop=(mq == MT - 1))
                nc.vector.tensor_copy(ob[:, bass.ts(ec, 512)], ps[:, 0:512])
            nc.sync.dma_start(out_d[bass.ts(qt, 128), :], ob[:])

        def wo_dma(_t):
            nc.sync.dma_start(
                wo_sb[:], wo_d.rearrange("(mt p) e -> p mt e", p=128))
            return
            yield

        # ---- weave scheduler ----
        # One generator step per attention kt (two while the K/V chunks —
        # hard deps of the first attention blocks — are still pending);
        # x-chunk DMA issued up to 2 items ahead of compute, in need order.
        # Each item carries the earliest attention block (qb*MT+m) it may
        # compute in, deferring Q-proj and out-proj into scalar-paced
        # windows.  item: (src_i, arg, factory, earliest_block)
        items = [
            (1, 1, lambda t: qk_proj_compute(t, 1, 1), 0),   # K chunk 1
            (1, 2, lambda t: qk_proj_compute(t, 1, 2), 0),   # K chunk 2
            (2, 2, lambda t: v_proj_compute(t, 2), 0),       # V st 8..11
            (1, 3, lambda t: qk_proj_compute(t, 1, 3), 0),   # K chunk 3
            (2, 3, lambda t: v_proj_compute(t, 3), 0),       # V st 12..15
            (0, 1, lambda t: qk_proj_compute(t, 0, 1), 2),   # Q chunk 1
            (None, 0, wo_dma, 2),
            (0, 2, lambda t: qk_proj_compute(t, 0, 2), 3),   # Q chunk 2
            (0, 3, lambda t: qk_proj_compute(t, 0, 3), 4),   # Q chunk 3
        ]
        N_KV_ITEMS = 5
        items += [(None, qt, lambda _t, qt=qt: outproj_group(qt),
                   (qt // 4 + 1) * MT)
                  for qt in range(12)]

        wv_state = {"pi": 0, "gi": 0, "tiles": {}, "gen": None}

        def weave_prefetch():
            pi = wv_state["pi"]
            if pi < len(items) and pi - wv_state["gi"] < 3:
                src_i, arg, _, _ = items[pi]
                wv_state["tiles"][pi] = (x_dma(src_i, arg)
                                         if src_i is not None else None)
                wv_state["pi"] = pi + 1

        def weave_step(block):
            st = wv_state
            if (st["gen"] is None and st["gi"] < st["pi"]
                    and items[st["gi"]][3] <= block):
                st["gen"] = items[st["gi"]][2](st["tiles"].pop(st["gi"]))
            if st["gen"] is not None:
                try:
                    next(st["gen"])
                except StopIteration:
                    st["gen"] = None
                    st["gi"] += 1
            weave_prefetch()

        def run_whole(gen):
            for _ in gen:
                pass

        # ---- prologue: chunk-0 K/Q projections (all m) + V st 0..7, DMAs
        # issued in first-use order so the first energy matmul isn't stuck
        # behind late-needed weights ----
        xk0 = x_dma(1, 0)
        nc.sync.dma_start(wq_sb[:], wq_d.rearrange("(kt p) m -> p kt m", p=128))
        nc.sync.dma_start(bq_sb[:], bq_d)
        xq0 = x_dma(0, 0)
        nc.sync.dma_start(wv_sb[:], wv_d.rearrange("(kt p) m -> p kt m", p=128))
        nc.sync.dma_start(bv_row[:], bv_d)
        # warm the exp table set while DMAs stream (one-time ~2.7us load)
        warm_in = smp.tile([1, 8], F32, tag="warm_i", name="warm_i")
        warm = smp.tile([1, 8], BF16, tag="warm_o", name="warm_o")
        nc.vector.memset(warm_in[:], 0.0)
        nc.scalar.activation(warm[:], warm_in[:], EXP)
        nc.gpsimd.partition_broadcast(bv_bc[:], bv_row[:])
        xv0 = x_dma(2, 0)
        xv1 = x_dma(2, 1)
        run_whole(qk_proj_compute(xk0, 1, 0))
        run_whole(qk_proj_compute(xq0, 0, 0))
        run_whole(v_proj_compute(xv0, 0))
        run_whole(v_proj_compute(xv1, 1))
        weave_prefetch()          # DMA for first woven item (K chunk 1)

        # ---- attention, with next m's Q/K projections woven in ----
        # Heads 2m / 2m+1 live on partitions 0:64 / 64:128 of QT/KT tile m,
        # so their energy matmuls auto-derive PE tile_position (0,0)/(64,0)
        # and run concurrently (row tiling) when issued back to back.  One
        # pe tile holds both heads' energies for a 512-wide q block, so one
        # exp activation covers the pair.
        for qb in range(4):                      # q blocks of 512
            for m in range(MT):
                q0 = qb * 512
                block = qb * MT + m
                po0 = po_pool.tile([HD + 1, 512], F32, tag="po", name="po0")
                po1 = po_pool.tile([HD + 1, 512], F32, tag="po", name="po1")
                pending = None
                for kt in range(ST):
                    pe = pe_pool.tile([128, 1024], F32, tag="pe")
                    for hs in range(2):
                        psl = slice(64 * hs, 64 * hs + 64)
                        nc.tensor.matmul(
                            pe[:, bass.ts(hs, 512)],
                            KT_sb[psl, m, bass.ts(kt, 128)],
                            QT_sb[psl, m, bass.ds(q0, 512)],
                            start=True, stop=True)
                    at = atp.tile([128, 1024], BF16, tag="at")
                    nc.scalar.activation(at[:], pe[:], EXP)
                    if pending is not None:
                        pkt, pat = pending
                        for hs, po in ((0, po0), (1, po1)):
                            nc.tensor.matmul(
                                po[:, :], V_sb[:, pkt, 2 * m + hs, :],
                                pat[:, bass.ts(hs, 512)],
                                start=(pkt == 0), stop=(pkt == ST - 1))
                    pending = (kt, at)
                    weave_step(block)
                    if wv_state["gi"] < N_KV_ITEMS:
                        weave_step(block)   # K/V chunks pending: drain fast
                pkt, pat = pending
                for hs, po in ((0, po0), (1, po1)):
                    nc.tensor.matmul(
                        po[:, :], V_sb[:, pkt, 2 * m + hs, :],
                        pat[:, bass.ts(hs, 512)],
                        start=(pkt == 0), stop=(pkt == ST - 1))

                # ---- normalize + evict ----
                # 1/sums straight out of PSUM row 64 into partition 0 of a
                # fresh SBUF tile (partition_broadcast ucode reads the
                # physical first partition of its input).
                for hs, po in ((0, po0), (1, po1)):
                    s_sb = smp.tile([1, 512], F32, tag="s")
                    nc.vector.tensor_copy(s_sb[0:1, :], po[HD:HD + 1, :])
                    nc.vector.reciprocal_approx_fast(s_sb[0:1, :], s_sb[0:1, :])
                    bc = smp.tile([HD, 512], F32, tag="bc")
                    nc.gpsimd.partition_broadcast(bc[:], s_sb[0:1, :])
                    nc.vector.tensor_tensor(
                        oT_sb[64 * hs:64 * hs + HD, m, bass.ds(q0, 512)],
                        po[0:HD, :], bc[:], mybir.AluOpType.mult)

        if dump:
            d_qt = nc.dram_tensor("d_qt", [128, MT, S], BF16, kind="ExternalOutput").ap()
            d_kt = nc.dram_tensor("d_kt", [128, MT, S], BF16, kind="ExternalOutput").ap()
            d_v = nc.dram_tensor("d_v", [128, ST, NHL, HD + 1], BF16, kind="ExternalOutput").ap()
            d_ot = nc.dram_tensor("d_ot", [128, MT, S], BF16, kind="ExternalOutput").ap()
            nc.sync.dma_start(d_qt, QT_sb[:])
            nc.sync.dma_start(d_kt, KT_sb[:])
            nc.sync.dma_start(d_v, V_sb[:])
            nc.sync.dma_start(d_ot, oT_sb[:])

        # ---- tail: remaining woven groups + final out-projection ----
        while wv_state["gi"] < len(items):
            weave_step(10 ** 9)
        for qt in range(12, ST):
            run_whole(outproj_group(qt))


_CACHED = {}


def _get_bass():
    if "nc" not in _CACHED:
        _CACHED["nc"] = _build_bass()
    return _CACHED["nc"]


def _prep_core_inputs(c, query, key, value, Wq, bq, Wk, bk, Wv, bv, Wo):
    b, half = c // 2, c % 2
    sl = slice(DLOC * half, DLOC * half + DLOC)
    bq_sl = (bq[sl] * 0.125).astype(np.float32).reshape(MT, 128).T.copy()
    bk_sl = bk[sl].astype(np.float32).reshape(MT, 128).T.copy()
    return {
        "xqT": np.ascontiguousarray(query[b].T).astype(NPBF),
        "xkT": np.ascontiguousarray(key[b].T).astype(NPBF),
        "xvT": np.ascontiguousarray(value[b].T).astype(NPBF),
        "wq": np.ascontiguousarray(Wq[sl, :].T * 0.125).astype(NPBF),
        "wk": np.ascontiguousarray(Wk[sl, :].T).astype(NPBF),
        "wv": np.ascontiguousarray(Wv[sl, :].T).astype(NPBF),
        "wo": np.ascontiguousarray(Wo[:, sl].T).astype(NPBF),
        "bq": np.ascontiguousarray(bq_sl),
        "bk": np.ascontiguousarray(bk_sl),
        "bv": bv[sl].astype(np.float32).reshape(1, DLOC).copy(),
    }


def kernel(query, key, value, Wq, bq, Wk, bk, Wv, bv, Wo, bo,
           trace=False, **run_kwargs):
    query = np.asarray(query, np.float32)
    key = np.asarray(key, np.float32)
    value = np.asarray(value, np.float32)
    Wq, Wk, Wv, Wo = (np.asarray(w, np.float32) for w in (Wq, Wk, Wv, Wo))
    bq, bk, bv, bo = (np.asarray(x, np.float32) for x in (bq, bk, bv, bo))

    nc = _get_bass()
    in_maps = [_prep_core_inputs(c, query, key, value, Wq, bq, Wk, bk, Wv, bv, Wo)
               for c in range(8)]
    res = run_bass_kernel_spmd(nc, in_maps, core_ids=list(range(8)),
                               trace=trace, **run_kwargs)
    _CACHED["last_result"] = res

    B = query.shape[0]
    out = np.empty((B, S, E), np.float32)
    for b in range(B):
        out[b] = res.results[2 * b]["out"] + res.results[2 * b + 1]["out"] + bo
    return out

